# revision 10
# baseline (speedup 1.0000x reference)
"""Multi-head self-attention (B=4, S=2048, D=768, H=12) on 8 Trainium2 cores.

v6: collective-free, key-compacted, engine-balanced.

Sharding: core (b, g) owns batch b, query rows [g*1024, (g+1)*1024), all 12
heads. Every core uploads the full weight pack and the x shards it needs, so
there are no on-device collectives (no global barrier, no launch-skew
sensitivity, no AllGather latency).

Key compaction: masked keys (mask==0) contribute exactly zero to softmax
numerator and denominator (exp(-1e9*s) == 0 in fp32), so the host drops them
before upload. The key sequence shrinks from 2048 to KVP = ceil(maxL/128)*128
(1152 for the seed-0 mask), cutting the k/v projections, score matmuls, exps
and PV matmuls by ~44%. Pad columns are zeros with mask=0 (their exp bias
forces exact-zero attention weight).

Engine plan per core:
  PE      q/k/v projections (fp8e3 operands straight from transport; the old
          bf16 upconvert was numerically exact so skipping it is free),
          scoresT (bf16), PV (bf16), out-proj (bf16).
  Scalar  exp activations (the structural bottleneck: S_kv*S_q*H elements).
  Pool    psum->sbuf casts for qT/kT, v' copies (keeps DVE/Scalar free).
  DVE     softmax denominator reciprocal (reciprocal_approx_fast) and the
          fused normalize-multiply op->attT.
  DMA     input staging, output writeback, denominator partition-broadcast.

Attention is software-pipelined per head: scores(h)+exp(h) are emitted before
PV(h-1), so the PE's in-order queue never parks behind an exp it doesn't need.

Numerics match the v5 baseline (rel err ~1.3e-2 vs the 2e-2 gate): x/W travel
as fp8 e3m4 (W scaled x256; the factor cancels in softmax normalization and
is divided out of the output on the host), bf16 matmuls with fp32 PSUM,
output partial as fp8 e3m4 x128, bv's rank-1 contribution added on host.
"""

import math

import numpy as np

import concourse.bass as bass
import concourse.mybir as mybir
import concourse.tile as tile
from concourse.bass_utils import run_bass_kernel_spmd

F32 = mybir.dt.float32
BF16 = mybir.dt.bfloat16
F8 = mybir.dt.float8e3

AF = mybir.ActivationFunctionType
ALU = mybir.AluOpType

D_MODEL = 768
NUM_HEADS = 12
D_QKV = 64
B = 4
S = 2048
SH = S // 2                 # per-core query rows
N_CORES = 8
KB_D = D_MODEL // 128       # 6 feature blocks

_PROGRAMS = {}              # KVP -> compiled Bass program


def _split_wide_waits(nc, max_waits=1):
    """walrus core_v3 codegen rejects >2 semaphore waits on one instruction.
    Hoist excess waits onto Drains inserted just before, on the same engine
    stream - sequential waits are equivalent."""
    for fn in nc.m.functions:
        for blk in fn.blocks:
            insts = blk.instructions
            i = 0
            while i < len(insts):
                inst = insts[i]
                si = inst.sync_info
                if si is not None and len(si.on_wait) > max_waits:
                    waits = list(si.on_wait)
                    keep, rest = waits[:max_waits], waits[max_waits:]
                    k = 0
                    while rest:
                        chunk, rest = rest[:max_waits], rest[max_waits:]
                        nop = mybir.InstDrain(
                            name=f"{inst.name}_wsplit{k}", ins=[], outs=[]
                        )
                        nop.engine = inst.engine
                        nop.is_reset_sema = False
                        nop.sync_info = mybir.SyncInfo(on_wait=chunk, on_update=[])
                        insts.insert(i, nop)
                        i += 1
                        k += 1
                    inst.sync_info = mybir.SyncInfo(
                        on_wait=keep, on_update=list(si.on_update)
                    )
                i += 1


def _build_program(KVP):
    KB = KVP // 128         # key partition-blocks
    nc = bass.Bass("TRN2", target_bir_lowering=False, debug=False)

    def din(name, shape, dt=F32):
        return nc.dram_tensor(name, list(shape), dt, kind="ExternalInput").ap()

    xq_d = din("xq", [D_MODEL, SH], F8)        # own query half, xT layout
    xkv_d = din("xkv", [D_MODEL, KVP], F8)     # compacted keys of this batch
    wp_d = din("wp", [4 * D_MODEL, D_MODEL], F8)   # [WqT;WkT;WvT;WoT] x256
    bqs_d = din("bqs", [128, KB_D])            # bq*256, [p, pb]
    bks_d = din("bks", [128, KB_D])
    sq_d = din("sq", [128, KB_D])              # scale/65536 per q feature
    kbias_d = din("kbias", [128, KB * NUM_HEADS])  # exp bias (0 / -1e9*s_h)
    out_d = nc.dram_tensor("out", [SH, D_MODEL], F8, kind="ExternalOutput").ap()

    with tile.TileContext(nc) as tc:
        with (
            tc.tile_pool(name="wpool", bufs=1) as wpool,
            tc.tile_pool(name="perp", bufs=1) as perp,
            tc.tile_pool(name="obp", bufs=2) as obp,
            tc.tile_pool(name="rbp", bufs=2) as rbp,
            tc.tile_pool(name="psp", bufs=1, space="PSUM") as psp,
        ):
            # ---- stage inputs in SBUF (fp8 kept as-is for the PE) --------
            xq = []
            for kb in range(KB_D):
                t = wpool.tile([128, SH], F8, name=f"xq{kb}", tag=f"xq{kb}")
                nc.sync.dma_start(out=t[:], in_=xq_d[kb * 128:(kb + 1) * 128, :])
                xq.append(t)
            xkv = []
            for kb in range(KB_D):
                t = wpool.tile([128, KVP], F8, name=f"xkv{kb}", tag=f"xkv{kb}")
                nc.sync.dma_start(out=t[:], in_=xkv_d[kb * 128:(kb + 1) * 128, :])
                xkv.append(t)

            def wtiles(base, pfx):
                ts = []
                for kb in range(KB_D):
                    t = wpool.tile([128, D_MODEL], F8, name=f"{pfx}{kb}",
                                   tag=f"{pfx}{kb}")
                    nc.sync.dma_start(
                        out=t[:],
                        in_=wp_d[base + kb * 128: base + (kb + 1) * 128, :])
                    ts.append(t)
                return ts

            wq = wtiles(0, "wq")
            wk = wtiles(D_MODEL, "wk")
            wv = wtiles(2 * D_MODEL, "wv")
            wo8 = wtiles(3 * D_MODEL, "wo8")
            # out-proj runs bf16 (attT is bf16): upconvert just Wo
            woT = []
            for pb in range(KB_D):
                t = wpool.tile([128, D_MODEL], BF16, name=f"woT{pb}",
                               tag=f"woT{pb}")
                nc.gpsimd.tensor_copy(t[:], wo8[pb][:])
                woT.append(t)

            onescol = wpool.tile([128, 64], F32, name="onescol", tag="onescol")
            nc.vector.memset(onescol[:], 1.0)
            bqs = wpool.tile([128, KB_D], F32, name="bqs", tag="bqs")
            bks = wpool.tile([128, KB_D], F32, name="bks", tag="bks")
            sq = wpool.tile([128, KB_D], F32, name="sq", tag="sq")
            kbias = wpool.tile([128, KB * NUM_HEADS], F32, name="kbias",
                               tag="kbias")
            nc.sync.dma_start(out=bqs[:], in_=bqs_d)
            nc.sync.dma_start(out=bks[:], in_=bks_d)
            nc.sync.dma_start(out=sq[:], in_=sq_d)
            nc.sync.dma_start(out=kbias[:], in_=kbias_d)

            qT = [perp.tile([128, SH], BF16, name=f"qT{pb}", tag=f"qT{pb}")
                  for pb in range(KB_D)]
            kT = [perp.tile([128, KVP], BF16, name=f"kT{pb}", tag=f"kT{pb}")
                  for pb in range(KB_D)]
            vp = [perp.tile([128, NUM_HEADS * 65], BF16, name=f"vp{sb}",
                            tag=f"vp{sb}")
                  for sb in range(KB)]
            attT = [perp.tile([128, SH], BF16, name=f"attT{pb}",
                              tag=f"attT{pb}")
                    for pb in range(KB_D)]
            # pt: exp(score) tiles; 2 heads in flight
            pt = [[perp.tile([128, SH], BF16, name=f"pt{s}_{kb}",
                             tag=f"pt{s}_{kb}")
                   for kb in range(KB)] for s in range(2)]

            # ---- phase 1: qT = (wqT.T @ xq + bq) * s ---------------------
            for pb in range(KB_D):
                ps = psp.tile([128, SH], F32, name="mmq", tag="sc", bufs=2)
                for kb in range(KB_D):
                    for nb in range(2):
                        nc.tensor.matmul(
                            ps[:, nb * 512:(nb + 1) * 512],
                            lhsT=wq[kb][:, pb * 128:(pb + 1) * 128],
                            rhs=xq[kb][:, nb * 512:(nb + 1) * 512],
                            start=(kb == 0),
                            stop=(kb == KB_D - 1),
                        )
                nc.vector.tensor_scalar(
                    out=qT[pb][:],
                    in0=ps[:],
                    scalar1=bqs[:, pb:pb + 1],
                    scalar2=sq[:, pb:pb + 1],
                    op0=ALU.add,
                    op1=ALU.mult,
                )

            # ---- phase 2: kT = wkT.T @ xkv + bk --------------------------
            kchunks = []
            off = 0
            while off < KVP:
                w = min(1024, KVP - off)
                kchunks.append((off, w))
                off += w
            for pb in range(KB_D):
                for off, w in kchunks:
                    ps = psp.tile([128, SH], F32, name="mmk", tag="sc", bufs=2)
                    for kb in range(KB_D):
                        c = 0
                        while c < w:
                            cw = min(512, w - c)
                            nc.tensor.matmul(
                                ps[:, c:c + cw],
                                lhsT=wk[kb][:, pb * 128:(pb + 1) * 128],
                                rhs=xkv[kb][:, off + c:off + c + cw],
                                start=(kb == 0),
                                stop=(kb == KB_D - 1),
                            )
                            c += cw
                    nc.vector.tensor_scalar(
                        out=kT[pb][:, off:off + w],
                        in0=ps[:, :w],
                        scalar1=bks[:, pb:pb + 1],
                        scalar2=None,
                        op0=ALU.add,
                    )

            # ---- phase 3: v' = [x @ wvT | 256] ---------------------------
            for sb in range(KB):
                ps = psp.tile([128, SH], F32, name="mmv", tag="sc", bufs=2)
                for kb in range(KB_D):
                    for lo, hi in ((0, 512), (512, 768)):
                        nc.tensor.matmul(
                            ps[:, lo:hi],
                            lhsT=xkv[kb][:, sb * 128:(sb + 1) * 128],
                            rhs=wv[kb][:, lo:hi],
                            start=(kb == 0),
                            stop=(kb == KB_D - 1),
                        )
                dst = vp[sb].rearrange("p (h c) -> p h c", c=65)[:, :, 0:64]
                nc.scalar.copy(
                    dst, ps[:, :D_MODEL].rearrange("p (h c) -> p h c", c=64))
                ones_col = vp[sb].rearrange("p (h c) -> p h c", c=65)[:, :, 64:65]
                nc.vector.memset(ones_col, 256.0)

            # ---- phase 4: attention, software-pipelined per head ---------
            op_tiles = [None, None]   # live op psum per pipeline slot

            def emit_scores(h):
                s = h % 2
                pb, po = h // 2, 64 * (h % 2)
                for kb in range(KB):
                    sc = psp.tile([128, SH], F32, name="sc", tag="sc", bufs=2)
                    for nb in range(2):
                        nc.tensor.matmul(
                            sc[:, nb * 512:(nb + 1) * 512],
                            lhsT=kT[pb][po:po + 64, kb * 128:(kb + 1) * 128],
                            rhs=qT[pb][po:po + 64, nb * 512:(nb + 1) * 512],
                            start=True,
                            stop=True,
                        )
                    nc.scalar.activation(
                        pt[s][kb][:],
                        sc[:],
                        AF.Exp,
                        bias=kbias[:, kb * NUM_HEADS + h:kb * NUM_HEADS + h + 1],
                        scale=1.0,
                    )

            def emit_pv(h):
                s = h % 2
                pb, po = h // 2, 64 * (h % 2)
                op = psp.tile([65, SH], F32, name="op", tag="op", bufs=2)
                for kb in range(KB):
                    for nb in range(2):
                        nc.tensor.matmul(
                            op[:, nb * 512:(nb + 1) * 512],
                            lhsT=vp[kb][:, h * 65:h * 65 + 65],
                            rhs=pt[s][kb][:, nb * 512:(nb + 1) * 512],
                            start=(kb == 0),
                            stop=(kb == KB - 1),
                        )
                # normalize: 1/den -> K=1 matmul broadcast into a bc psum
                # slot borrowed from the "op" tag -> fused multiply.
                rrow = rbp.tile([1, SH], F32, name="rrow", tag="rrow", bufs=2)
                nc.vector.reciprocal(rrow[:], op[64:65, :])
                att_u = rbp.tile([64, SH], BF16, name="att_u", tag="att_u",
                                 bufs=2)
                nc.vector.tensor_copy(att_u[:], op[0:64, :])
                bc = psp.tile([128, SH], F32, name="bc", tag="sc", bufs=2)
                for nb in range(2):
                    nc.tensor.matmul(
                        bc[0:64, nb * 512:(nb + 1) * 512],
                        lhsT=onescol[0:1, 0:64],
                        rhs=rrow[0:1, nb * 512:(nb + 1) * 512],
                        start=True,
                        stop=True,
                    )
                if po == 0:
                    nc.vector.tensor_mul(
                        attT[pb][0:64, :], att_u[:], bc[0:64, :])
                else:
                    stage = rbp.tile([64, SH], BF16, name="nstage",
                                     tag="nstage", bufs=2)
                    nc.vector.tensor_mul(stage[:], att_u[:], bc[0:64, :])
                    nc.sync.dma_start(out=attT[pb][64:128, :], in_=stage[:])

            emit_scores(0)
            for h in range(1, NUM_HEADS):
                emit_scores(h)
                emit_pv(h - 1)
            emit_pv(NUM_HEADS - 1)

            # ---- phase 5: out = attT.T @ woT -----------------------------
            for sb in range(SH // 128):
                ps = psp.tile([128, SH], F32, name="mmo", tag="sc", bufs=2)
                for pb in range(KB_D):
                    for lo, hi in ((0, 512), (512, 768)):
                        nc.tensor.matmul(
                            ps[:, lo:hi],
                            lhsT=attT[pb][:, sb * 128:(sb + 1) * 128],
                            rhs=woT[pb][:, lo:hi],
                            start=(pb == 0),
                            stop=(pb == KB_D - 1),
                        )
                ob = obp.tile([128, D_MODEL], F8, name="ob", tag="ob")
                # psum = att * (256*Wo) = 256*out_true; store 128*out_true
                nc.scalar.activation(ob[:], ps[:, :D_MODEL], AF.Identity,
                                     bias=0.0, scale=0.5)
                nc.sync.dma_start(
                    out=out_d[sb * 128:(sb + 1) * 128, :], in_=ob[:])

    _split_wide_waits(nc)
    return nc


def _plan_kvp(mask):
    counts = [int((mask[b] != 0).sum()) for b in range(B)]
    kvp = max(128, int(math.ceil(max(counts) / 128.0)) * 128)
    return min(kvp, S)


def _prep_core_inputs(x, mask, Wq, bq, Wk, bk, Wv, bv, Wo, bo, temperature,
                      KVP):
    """Build the 8 per-core input dicts (disjoint fp8 shards, no gathers)."""
    import ml_dtypes

    f8 = ml_dtypes.float8_e3m4
    KB = KVP // 128
    scale = (np.asarray(temperature, np.float64)
             / math.sqrt(D_QKV)).astype(np.float32)       # [12]

    pack = (np.concatenate([Wq.T, Wk.T, Wv.T, Wo.T], axis=0) * 256).astype(f8)

    pidx = np.arange(128)
    bqs = (256.0 * bq.reshape(KB_D, 128).T).astype(np.float32)
    bqs = np.ascontiguousarray(bqs)                       # [128, 6]
    bks = np.ascontiguousarray(
        (256.0 * bk.reshape(KB_D, 128).T).astype(np.float32))
    # feature f = pb*128 + p belongs to head f//64
    heads = (pidx[:, None] + 128 * np.arange(KB_D)[None, :]) // D_QKV
    sqm = np.ascontiguousarray(
        (scale[heads] / 65536.0).astype(np.float32))      # [128, 6]

    in_maps = []
    per_batch = {}
    for b in range(B):
        live = np.nonzero(np.asarray(mask[b]) != 0)[0]
        xkv = np.zeros((D_MODEL, KVP), np.float32)
        xkv[:, :live.size] = x[b].T[:, live]
        kmask = np.zeros(KVP, np.float32)
        kmask[:live.size] = 1.0
        # kbias[p, kb*12 + h] = (kmask-1) * 1e9 * scale[h]
        km = kmask.reshape(KB, 128)                       # [KB, 128]
        kbias = ((km[:, :, None] - 1.0) * (1e9 * scale)[None, None, :])
        kbias = np.ascontiguousarray(
            kbias.transpose(1, 0, 2).reshape(128, KB * NUM_HEADS)
        ).astype(np.float32)
        per_batch[b] = (xkv.astype(f8), kbias)

    for core in range(N_CORES):
        b, g = core // 2, core % 2
        xkv8, kbias = per_batch[b]
        in_maps.append({
            "xq": np.ascontiguousarray(
                x[b].T[:, g * SH:(g + 1) * SH]).astype(f8),
            "xkv": xkv8,
            "wp": pack,
            "bqs": bqs, "bks": bks, "sq": sqm,
            "kbias": kbias,
        })
    return in_maps


def kernel(x, mask, Wq, bq, Wk, bk, Wv, bv, Wo, bo, temperature, **kw):
    x = np.asarray(x, np.float32)
    mask = np.asarray(mask)
    args = [np.asarray(a, np.float32) for a in (Wq, bq, Wk, bk, Wv, bv, Wo, bo)]
    temperature = np.asarray(temperature, np.float32)

    KVP = _plan_kvp(mask)
    if KVP not in _PROGRAMS:
        _PROGRAMS[KVP] = _build_program(KVP)
    nc = _PROGRAMS[KVP]

    in_maps = _prep_core_inputs(x, mask, *args, temperature, KVP)
    res = run_bass_kernel_spmd(nc, in_maps, core_ids=list(range(N_CORES)))

    Wo_f, bo_f, bv_f = args[6], args[7], args[5]
    hostvec = bv_f @ Wo_f.T + bo_f   # bv contributes a fixed row vector
    out = np.empty((B, S, D_MODEL), np.float32)
    for b in range(B):
        for g in range(2):
            out[b, g * SH:(g + 1) * SH] = (
                res.results[2 * b + g]["out"].astype(np.float32) * (1 / 128)
                + hostvec)
    return out


# revision 12
# speedup vs baseline: 1.0109x; 1.0109x over previous
"""Multi-head self-attention (B=4, S=2048, D=768, H=12) on 8 Trainium2 cores.

v6: collective-free, key-compacted, engine-balanced.

Sharding: core (b, g) owns batch b, query rows [g*1024, (g+1)*1024), all 12
heads. Every core uploads the full weight pack and the x shards it needs, so
there are no on-device collectives (no global barrier, no launch-skew
sensitivity, no AllGather latency).

Key compaction: masked keys (mask==0) contribute exactly zero to softmax
numerator and denominator (exp(-1e9*s) == 0 in fp32), so the host drops them
before upload. The key sequence shrinks from 2048 to KVP = ceil(maxL/128)*128
(1152 for the seed-0 mask), cutting the k/v projections, score matmuls, exps
and PV matmuls by ~44%. Pad columns are zeros with mask=0 (their exp bias
forces exact-zero attention weight).

Engine plan per core:
  PE      q/k/v projections (fp8e3 operands straight from transport; the old
          bf16 upconvert was numerically exact so skipping it is free),
          scoresT (bf16), PV (bf16), out-proj (bf16).
  Scalar  exp activations (the structural bottleneck: S_kv*S_q*H elements).
  Pool    psum->sbuf casts for qT/kT, v' copies (keeps DVE/Scalar free).
  DVE     softmax denominator reciprocal (reciprocal_approx_fast) and the
          fused normalize-multiply op->attT.
  DMA     input staging, output writeback, denominator partition-broadcast.

Attention is software-pipelined per head: scores(h)+exp(h) are emitted before
PV(h-1), so the PE's in-order queue never parks behind an exp it doesn't need.

Numerics match the v5 baseline (rel err ~1.3e-2 vs the 2e-2 gate): x/W travel
as fp8 e3m4 (W scaled x256; the factor cancels in softmax normalization and
is divided out of the output on the host), bf16 matmuls with fp32 PSUM,
output partial as fp8 e3m4 x128, bv's rank-1 contribution added on host.
"""

import math

import numpy as np

import concourse.bass as bass
import concourse.mybir as mybir
import concourse.tile as tile
from concourse.bass_utils import run_bass_kernel_spmd

F32 = mybir.dt.float32
BF16 = mybir.dt.bfloat16
F8 = mybir.dt.float8e3

AF = mybir.ActivationFunctionType
ALU = mybir.AluOpType

D_MODEL = 768
NUM_HEADS = 12
D_QKV = 64
B = 4
S = 2048
SH = S // 2                 # per-core query rows
N_CORES = 8
KB_D = D_MODEL // 128       # 6 feature blocks

_PROGRAMS = {}              # KVP -> compiled Bass program


def _split_wide_waits(nc, max_waits=1):
    """walrus core_v3 codegen rejects >2 semaphore waits on one instruction.
    Hoist excess waits onto Drains inserted just before, on the same engine
    stream - sequential waits are equivalent."""
    for fn in nc.m.functions:
        for blk in fn.blocks:
            insts = blk.instructions
            i = 0
            while i < len(insts):
                inst = insts[i]
                si = inst.sync_info
                if si is not None and len(si.on_wait) > max_waits:
                    waits = list(si.on_wait)
                    keep, rest = waits[:max_waits], waits[max_waits:]
                    k = 0
                    while rest:
                        chunk, rest = rest[:max_waits], rest[max_waits:]
                        nop = mybir.InstDrain(
                            name=f"{inst.name}_wsplit{k}", ins=[], outs=[]
                        )
                        nop.engine = inst.engine
                        nop.is_reset_sema = False
                        nop.sync_info = mybir.SyncInfo(on_wait=chunk, on_update=[])
                        insts.insert(i, nop)
                        i += 1
                        k += 1
                    inst.sync_info = mybir.SyncInfo(
                        on_wait=keep, on_update=list(si.on_update)
                    )
                i += 1


def _build_program(KVP):
    KB = KVP // 128         # key partition-blocks
    nc = bass.Bass("TRN2", target_bir_lowering=False, debug=False)

    def din(name, shape, dt=F32):
        return nc.dram_tensor(name, list(shape), dt, kind="ExternalInput").ap()

    xq_d = din("xq", [D_MODEL, SH], F8)        # own query half, xT layout
    xkv_d = din("xkv", [D_MODEL, KVP], F8)     # compacted keys of this batch
    wp_d = din("wp", [4 * D_MODEL, D_MODEL], F8)   # [WqT;WkT;WvT;WoT] x256
    bqs_d = din("bqs", [128, KB_D])            # bq*256, [p, pb]
    bks_d = din("bks", [128, KB_D])
    sq_d = din("sq", [128, KB_D])              # scale/65536 per q feature
    kbias_d = din("kbias", [128, KB * NUM_HEADS])  # exp bias (0 / -1e9*s_h)
    out_d = nc.dram_tensor("out", [SH, D_MODEL], F8, kind="ExternalOutput").ap()

    with tile.TileContext(nc) as tc:
        with (
            tc.tile_pool(name="wpool", bufs=1) as wpool,
            tc.tile_pool(name="perp", bufs=1) as perp,
            tc.tile_pool(name="obp", bufs=2) as obp,
            tc.tile_pool(name="rbp", bufs=2) as rbp,
            tc.tile_pool(name="psp", bufs=1, space="PSUM") as psp,
        ):
            # ---- stage inputs in SBUF (fp8 kept as-is for the PE) --------
            def wtiles(base, pfx):
                ts = []
                for kb in range(KB_D):
                    t = wpool.tile([128, D_MODEL], F8, name=f"{pfx}{kb}",
                                   tag=f"{pfx}{kb}")
                    nc.sync.dma_start(
                        out=t[:],
                        in_=wp_d[base + kb * 128: base + (kb + 1) * 128, :])
                    ts.append(t)
                return ts

            # DMA order = consumption order: q-proj (wq,xq) can start while
            # the rest of the input set is still streaming in.
            wq = wtiles(0, "wq")
            xq = []
            for kb in range(KB_D):
                t = wpool.tile([128, SH], F8, name=f"xq{kb}", tag=f"xq{kb}")
                nc.sync.dma_start(out=t[:], in_=xq_d[kb * 128:(kb + 1) * 128, :])
                xq.append(t)
            wk = wtiles(D_MODEL, "wk")
            xkv = []
            for kb in range(KB_D):
                t = wpool.tile([128, KVP], F8, name=f"xkv{kb}", tag=f"xkv{kb}")
                nc.sync.dma_start(out=t[:], in_=xkv_d[kb * 128:(kb + 1) * 128, :])
                xkv.append(t)
            wv = wtiles(2 * D_MODEL, "wv")
            wo8 = wtiles(3 * D_MODEL, "wo8")
            # out-proj runs bf16 (attT is bf16): upconvert just Wo
            woT = []
            for pb in range(KB_D):
                t = wpool.tile([128, D_MODEL], BF16, name=f"woT{pb}",
                               tag=f"woT{pb}")
                nc.gpsimd.tensor_copy(t[:], wo8[pb][:])
                woT.append(t)

            onescol = wpool.tile([128, 64], F32, name="onescol", tag="onescol")
            nc.vector.memset(onescol[:], 1.0)
            bqs = wpool.tile([128, KB_D], F32, name="bqs", tag="bqs")
            bks = wpool.tile([128, KB_D], F32, name="bks", tag="bks")
            sq = wpool.tile([128, KB_D], F32, name="sq", tag="sq")
            kbias = wpool.tile([128, KB * NUM_HEADS], F32, name="kbias",
                               tag="kbias")
            nc.sync.dma_start(out=bqs[:], in_=bqs_d)
            nc.sync.dma_start(out=bks[:], in_=bks_d)
            nc.sync.dma_start(out=sq[:], in_=sq_d)
            nc.sync.dma_start(out=kbias[:], in_=kbias_d)

            qT = [perp.tile([128, SH], BF16, name=f"qT{pb}", tag=f"qT{pb}")
                  for pb in range(KB_D)]
            kT = [perp.tile([128, KVP], BF16, name=f"kT{pb}", tag=f"kT{pb}")
                  for pb in range(KB_D)]
            vp = [perp.tile([128, NUM_HEADS * 65], BF16, name=f"vp{sb}",
                            tag=f"vp{sb}")
                  for sb in range(KB)]
            attT = [perp.tile([128, SH], BF16, name=f"attT{pb}",
                              tag=f"attT{pb}")
                    for pb in range(KB_D)]
            # pt: exp(score) tiles; 2 heads in flight
            pt = [[perp.tile([128, SH], BF16, name=f"pt{s}_{kb}",
                             tag=f"pt{s}_{kb}")
                   for kb in range(KB)] for s in range(2)]

            # ---- phase 1: qT = (wqT.T @ xq + bq) * s ---------------------
            for pb in range(KB_D):
                ps = psp.tile([128, SH], F32, name="mmq", tag="sc", bufs=2)
                for kb in range(KB_D):
                    for nb in range(2):
                        nc.tensor.matmul(
                            ps[:, nb * 512:(nb + 1) * 512],
                            lhsT=wq[kb][:, pb * 128:(pb + 1) * 128],
                            rhs=xq[kb][:, nb * 512:(nb + 1) * 512],
                            start=(kb == 0),
                            stop=(kb == KB_D - 1),
                        )
                nc.vector.tensor_scalar(
                    out=qT[pb][:],
                    in0=ps[:],
                    scalar1=bqs[:, pb:pb + 1],
                    scalar2=sq[:, pb:pb + 1],
                    op0=ALU.add,
                    op1=ALU.mult,
                )

            # ---- phase 2: kT = wkT.T @ xkv + bk --------------------------
            kchunks = []
            off = 0
            while off < KVP:
                w = min(1024, KVP - off)
                kchunks.append((off, w))
                off += w
            for pb in range(KB_D):
                for off, w in kchunks:
                    ps = psp.tile([128, SH], F32, name="mmk", tag="sc", bufs=2)
                    for kb in range(KB_D):
                        c = 0
                        while c < w:
                            cw = min(512, w - c)
                            nc.tensor.matmul(
                                ps[:, c:c + cw],
                                lhsT=wk[kb][:, pb * 128:(pb + 1) * 128],
                                rhs=xkv[kb][:, off + c:off + c + cw],
                                start=(kb == 0),
                                stop=(kb == KB_D - 1),
                            )
                            c += cw
                    nc.vector.tensor_scalar(
                        out=kT[pb][:, off:off + w],
                        in0=ps[:, :w],
                        scalar1=bks[:, pb:pb + 1],
                        scalar2=None,
                        op0=ALU.add,
                    )

            # ---- phase 3: v' = [x @ wvT | 256] ---------------------------
            for sb in range(KB):
                ps = psp.tile([128, SH], F32, name="mmv", tag="sc", bufs=2)
                for kb in range(KB_D):
                    for lo, hi in ((0, 512), (512, 768)):
                        nc.tensor.matmul(
                            ps[:, lo:hi],
                            lhsT=xkv[kb][:, sb * 128:(sb + 1) * 128],
                            rhs=wv[kb][:, lo:hi],
                            start=(kb == 0),
                            stop=(kb == KB_D - 1),
                        )
                dst = vp[sb].rearrange("p (h c) -> p h c", c=65)[:, :, 0:64]
                nc.scalar.copy(
                    dst, ps[:, :D_MODEL].rearrange("p (h c) -> p h c", c=64))
                ones_col = vp[sb].rearrange("p (h c) -> p h c", c=65)[:, :, 64:65]
                nc.vector.memset(ones_col, 256.0)

            # ---- phase 4: attention, software-pipelined per head ---------
            op_tiles = [None, None]   # live op psum per pipeline slot

            def emit_scores(h):
                s = h % 2
                pb, po = h // 2, 64 * (h % 2)
                for kb in range(KB):
                    sc = psp.tile([128, SH], F32, name="sc", tag="sc", bufs=2)
                    for nb in range(2):
                        nc.tensor.matmul(
                            sc[:, nb * 512:(nb + 1) * 512],
                            lhsT=kT[pb][po:po + 64, kb * 128:(kb + 1) * 128],
                            rhs=qT[pb][po:po + 64, nb * 512:(nb + 1) * 512],
                            start=True,
                            stop=True,
                        )
                    nc.scalar.activation(
                        pt[s][kb][:],
                        sc[:],
                        AF.Exp,
                        bias=kbias[:, kb * NUM_HEADS + h:kb * NUM_HEADS + h + 1],
                        scale=1.0,
                    )

            rrows = [None] * NUM_HEADS
            att_us = [None] * NUM_HEADS

            def emit_pv(h):
                s = h % 2
                op = psp.tile([65, SH], F32, name="op", tag="op", bufs=2)
                for kb in range(KB):
                    for nb in range(2):
                        nc.tensor.matmul(
                            op[:, nb * 512:(nb + 1) * 512],
                            lhsT=vp[kb][:, h * 65:h * 65 + 65],
                            rhs=pt[s][kb][:, nb * 512:(nb + 1) * 512],
                            start=(kb == 0),
                            stop=(kb == KB - 1),
                        )
                # drain op quickly (frees the psum slot): numerators to SBUF,
                # 1/den to a row tile. The broadcast+multiply happen one head
                # later so the PE never waits on the reciprocal.
                rrow = rbp.tile([1, SH], F32, name="rrow", tag="rrow", bufs=3)
                nc.vector.reciprocal(rrow[:], op[64:65, :])
                att_u = rbp.tile([64, SH], BF16, name="att_u", tag="att_u",
                                 bufs=3)
                nc.vector.tensor_copy(att_u[:], op[0:64, :])
                rrows[h], att_us[h] = rrow, att_u

            def emit_norm(h):
                pb, po = h // 2, 64 * (h % 2)
                bc = psp.tile([128, SH], F32, name="bc", tag="sc", bufs=2)
                for nb in range(2):
                    nc.tensor.matmul(
                        bc[0:64, nb * 512:(nb + 1) * 512],
                        lhsT=onescol[0:1, 0:64],
                        rhs=rrows[h][0:1, nb * 512:(nb + 1) * 512],
                        start=True,
                        stop=True,
                    )
                if po == 0:
                    nc.vector.tensor_mul(
                        attT[pb][0:64, :], att_us[h][:], bc[0:64, :])
                else:
                    stage = rbp.tile([64, SH], BF16, name="nstage",
                                     tag="nstage", bufs=2)
                    nc.vector.tensor_mul(stage[:], att_us[h][:], bc[0:64, :])
                    nc.sync.dma_start(out=attT[pb][64:128, :], in_=stage[:])

            emit_scores(0)
            emit_scores(1)
            emit_pv(0)
            for h in range(2, NUM_HEADS):
                emit_scores(h)
                emit_pv(h - 1)
                emit_norm(h - 2)
            emit_pv(NUM_HEADS - 1)
            emit_norm(NUM_HEADS - 2)
            emit_norm(NUM_HEADS - 1)

            # ---- phase 5: out = attT.T @ woT -----------------------------
            for sb in range(SH // 128):
                ps = psp.tile([128, SH], F32, name="mmo", tag="sc", bufs=2)
                for pb in range(KB_D):
                    for lo, hi in ((0, 512), (512, 768)):
                        nc.tensor.matmul(
                            ps[:, lo:hi],
                            lhsT=attT[pb][:, sb * 128:(sb + 1) * 128],
                            rhs=woT[pb][:, lo:hi],
                            start=(pb == 0),
                            stop=(pb == KB_D - 1),
                        )
                ob = obp.tile([128, D_MODEL], F8, name="ob", tag="ob")
                # psum = att * (256*Wo) = 256*out_true; store 128*out_true
                nc.scalar.activation(ob[:], ps[:, :D_MODEL], AF.Identity,
                                     bias=0.0, scale=0.5)
                nc.sync.dma_start(
                    out=out_d[sb * 128:(sb + 1) * 128, :], in_=ob[:])

    _split_wide_waits(nc)
    return nc


def _plan_kvp(mask):
    counts = [int((mask[b] != 0).sum()) for b in range(B)]
    kvp = max(128, int(math.ceil(max(counts) / 128.0)) * 128)
    return min(kvp, S)


def _prep_core_inputs(x, mask, Wq, bq, Wk, bk, Wv, bv, Wo, bo, temperature,
                      KVP):
    """Build the 8 per-core input dicts (disjoint fp8 shards, no gathers)."""
    import ml_dtypes

    f8 = ml_dtypes.float8_e3m4
    KB = KVP // 128
    scale = (np.asarray(temperature, np.float64)
             / math.sqrt(D_QKV)).astype(np.float32)       # [12]

    pack = (np.concatenate([Wq.T, Wk.T, Wv.T, Wo.T], axis=0) * 256).astype(f8)

    pidx = np.arange(128)
    bqs = (256.0 * bq.reshape(KB_D, 128).T).astype(np.float32)
    bqs = np.ascontiguousarray(bqs)                       # [128, 6]
    bks = np.ascontiguousarray(
        (256.0 * bk.reshape(KB_D, 128).T).astype(np.float32))
    # feature f = pb*128 + p belongs to head f//64
    heads = (pidx[:, None] + 128 * np.arange(KB_D)[None, :]) // D_QKV
    sqm = np.ascontiguousarray(
        (scale[heads] / 65536.0).astype(np.float32))      # [128, 6]

    in_maps = []
    per_batch = {}
    for b in range(B):
        live = np.nonzero(np.asarray(mask[b]) != 0)[0]
        xkv = np.zeros((D_MODEL, KVP), np.float32)
        xkv[:, :live.size] = x[b].T[:, live]
        kmask = np.zeros(KVP, np.float32)
        kmask[:live.size] = 1.0
        # kbias[p, kb*12 + h] = (kmask-1) * 1e9 * scale[h]
        km = kmask.reshape(KB, 128)                       # [KB, 128]
        kbias = ((km[:, :, None] - 1.0) * (1e9 * scale)[None, None, :])
        kbias = np.ascontiguousarray(
            kbias.transpose(1, 0, 2).reshape(128, KB * NUM_HEADS)
        ).astype(np.float32)
        per_batch[b] = (xkv.astype(f8), kbias)

    for core in range(N_CORES):
        b, g = core // 2, core % 2
        xkv8, kbias = per_batch[b]
        in_maps.append({
            "xq": np.ascontiguousarray(
                x[b].T[:, g * SH:(g + 1) * SH]).astype(f8),
            "xkv": xkv8,
            "wp": pack,
            "bqs": bqs, "bks": bks, "sq": sqm,
            "kbias": kbias,
        })
    return in_maps


def kernel(x, mask, Wq, bq, Wk, bk, Wv, bv, Wo, bo, temperature, **kw):
    x = np.asarray(x, np.float32)
    mask = np.asarray(mask)
    args = [np.asarray(a, np.float32) for a in (Wq, bq, Wk, bk, Wv, bv, Wo, bo)]
    temperature = np.asarray(temperature, np.float32)

    KVP = _plan_kvp(mask)
    if KVP not in _PROGRAMS:
        _PROGRAMS[KVP] = _build_program(KVP)
    nc = _PROGRAMS[KVP]

    in_maps = _prep_core_inputs(x, mask, *args, temperature, KVP)
    res = run_bass_kernel_spmd(nc, in_maps, core_ids=list(range(N_CORES)))

    Wo_f, bo_f, bv_f = args[6], args[7], args[5]
    hostvec = bv_f @ Wo_f.T + bo_f   # bv contributes a fixed row vector
    out = np.empty((B, S, D_MODEL), np.float32)
    for b in range(B):
        for g in range(2):
            out[b, g * SH:(g + 1) * SH] = (
                res.results[2 * b + g]["out"].astype(np.float32) * (1 / 128)
                + hostvec)
    return out


# revision 15
# speedup vs baseline: 1.1163x; 1.1042x over previous
"""Multi-head self-attention (B=4, S=2048, D=768, H=12) on 8 Trainium2 cores.

v6: collective-free, key-compacted, engine-balanced.

Sharding: core (b, g) owns batch b, query rows [g*1024, (g+1)*1024), all 12
heads. Every core uploads the full weight pack and the x shards it needs, so
there are no on-device collectives (no global barrier, no launch-skew
sensitivity, no AllGather latency).

Key compaction: masked keys (mask==0) contribute exactly zero to softmax
numerator and denominator (exp(-1e9*s) == 0 in fp32), so the host drops them
before upload. The key sequence shrinks from 2048 to KVP = ceil(maxL/128)*128
(1152 for the seed-0 mask), cutting the k/v projections, score matmuls, exps
and PV matmuls by ~44%. Pad columns are zeros with mask=0 (their exp bias
forces exact-zero attention weight).

Engine plan per core:
  PE      q/k/v projections (fp8e3 operands straight from transport; the old
          bf16 upconvert was numerically exact so skipping it is free),
          scoresT (bf16), PV (bf16), out-proj (bf16).
  Scalar  exp activations (the structural bottleneck: S_kv*S_q*H elements).
  Pool    psum->sbuf casts for qT/kT, v' copies (keeps DVE/Scalar free).
  DVE     softmax denominator reciprocal (reciprocal_approx_fast) and the
          fused normalize-multiply op->attT.
  DMA     input staging, output writeback, denominator partition-broadcast.

Attention is software-pipelined per head: scores(h)+exp(h) are emitted before
PV(h-1), so the PE's in-order queue never parks behind an exp it doesn't need.

Numerics match the v5 baseline (rel err ~1.3e-2 vs the 2e-2 gate): x/W travel
as fp8 e3m4 (W scaled x256; the factor cancels in softmax normalization and
is divided out of the output on the host), bf16 matmuls with fp32 PSUM,
output partial as fp8 e3m4 x128, bv's rank-1 contribution added on host.
"""

import math

import numpy as np

import concourse.bass as bass
import concourse.mybir as mybir
import concourse.tile as tile
from concourse.bass_utils import run_bass_kernel_spmd

F32 = mybir.dt.float32
BF16 = mybir.dt.bfloat16
F8 = mybir.dt.float8e3

AF = mybir.ActivationFunctionType
ALU = mybir.AluOpType

D_MODEL = 768
NUM_HEADS = 12
D_QKV = 64
B = 4
S = 2048
SH = S // 2                 # per-core query rows
N_CORES = 8
KB_D = D_MODEL // 128       # 6 feature blocks

_PROGRAMS = {}              # KVP -> compiled Bass program


def _split_wide_waits(nc, max_waits=1):
    """walrus core_v3 codegen rejects >2 semaphore waits on one instruction.
    Hoist excess waits onto Drains inserted just before, on the same engine
    stream - sequential waits are equivalent."""
    for fn in nc.m.functions:
        for blk in fn.blocks:
            insts = blk.instructions
            i = 0
            while i < len(insts):
                inst = insts[i]
                si = inst.sync_info
                if si is not None and len(si.on_wait) > max_waits:
                    waits = list(si.on_wait)
                    keep, rest = waits[:max_waits], waits[max_waits:]
                    k = 0
                    while rest:
                        chunk, rest = rest[:max_waits], rest[max_waits:]
                        nop = mybir.InstDrain(
                            name=f"{inst.name}_wsplit{k}", ins=[], outs=[]
                        )
                        nop.engine = inst.engine
                        nop.is_reset_sema = False
                        nop.sync_info = mybir.SyncInfo(on_wait=chunk, on_update=[])
                        insts.insert(i, nop)
                        i += 1
                        k += 1
                    inst.sync_info = mybir.SyncInfo(
                        on_wait=keep, on_update=list(si.on_update)
                    )
                i += 1


def _build_program(KVP):
    KB = KVP // 128         # key partition-blocks
    nc = bass.Bass("TRN2", target_bir_lowering=False, debug=False)

    def din(name, shape, dt=F32):
        return nc.dram_tensor(name, list(shape), dt, kind="ExternalInput").ap()

    xq_d = din("xq", [D_MODEL, SH], F8)        # own query half, xT layout
    xkv_d = din("xkv", [D_MODEL, KVP], F8)     # compacted keys of this batch
    wp_d = din("wp", [4 * D_MODEL, D_MODEL], F8)   # [WqT;WkT;WvT;WoT] x256
    bqs_d = din("bqs", [128, KB_D])            # bq*256, [p, pb]
    bks_d = din("bks", [128, KB_D])
    sq_d = din("sq", [128, KB_D])              # scale/65536 per q feature
    kbias_d = din("kbias", [128, KB * NUM_HEADS])  # exp bias (0 / -1e9*s_h)
    out_d = nc.dram_tensor("out", [SH, D_MODEL], F8, kind="ExternalOutput").ap()

    with tile.TileContext(nc) as tc:
        with (
            tc.tile_pool(name="wpool", bufs=1) as wpool,
            tc.tile_pool(name="perp", bufs=1) as perp,
            tc.tile_pool(name="obp", bufs=2) as obp,
            tc.tile_pool(name="rbp", bufs=2) as rbp,
            tc.tile_pool(name="psp", bufs=1, space="PSUM") as psp,
        ):
            # ---- stage inputs in SBUF (fp8 kept as-is for the PE) --------
            def wtiles(base, pfx):
                ts = []
                for kb in range(KB_D):
                    t = wpool.tile([128, D_MODEL], F8, name=f"{pfx}{kb}",
                                   tag=f"{pfx}{kb}")
                    nc.sync.dma_start(
                        out=t[:],
                        in_=wp_d[base + kb * 128: base + (kb + 1) * 128, :])
                    ts.append(t)
                return ts

            # DMA order = consumption order: q-proj (wq,xq) can start while
            # the rest of the input set is still streaming in.
            wq = wtiles(0, "wq")
            xq = []
            for kb in range(KB_D):
                t = wpool.tile([128, SH], F8, name=f"xq{kb}", tag=f"xq{kb}")
                nc.sync.dma_start(out=t[:], in_=xq_d[kb * 128:(kb + 1) * 128, :])
                xq.append(t)
            wk = wtiles(D_MODEL, "wk")
            xkv = []
            for kb in range(KB_D):
                t = wpool.tile([128, KVP], F8, name=f"xkv{kb}", tag=f"xkv{kb}")
                nc.sync.dma_start(out=t[:], in_=xkv_d[kb * 128:(kb + 1) * 128, :])
                xkv.append(t)
            wv = wtiles(2 * D_MODEL, "wv")
            wo8 = wtiles(3 * D_MODEL, "wo8")
            # out-proj runs bf16 (attT is bf16): upconvert just Wo
            woT = []
            for pb in range(KB_D):
                t = wpool.tile([128, D_MODEL], BF16, name=f"woT{pb}",
                               tag=f"woT{pb}")
                nc.gpsimd.tensor_copy(t[:], wo8[pb][:])
                woT.append(t)

            onescol = wpool.tile([128, 64], F32, name="onescol", tag="onescol")
            nc.vector.memset(onescol[:], 1.0)
            bqs = wpool.tile([128, KB_D], F32, name="bqs", tag="bqs")
            bks = wpool.tile([128, KB_D], F32, name="bks", tag="bks")
            sq = wpool.tile([128, KB_D], F32, name="sq", tag="sq")
            kbias = wpool.tile([128, KB * NUM_HEADS], F32, name="kbias",
                               tag="kbias")
            nc.sync.dma_start(out=bqs[:], in_=bqs_d)
            nc.sync.dma_start(out=bks[:], in_=bks_d)
            nc.sync.dma_start(out=sq[:], in_=sq_d)
            nc.sync.dma_start(out=kbias[:], in_=kbias_d)

            qT = [perp.tile([128, SH], BF16, name=f"qT{pb}", tag=f"qT{pb}")
                  for pb in range(KB_D)]
            kT = [perp.tile([128, KVP], BF16, name=f"kT{pb}", tag=f"kT{pb}")
                  for pb in range(KB_D)]
            vp = [perp.tile([128, NUM_HEADS * 65], BF16, name=f"vp{sb}",
                            tag=f"vp{sb}")
                  for sb in range(KB)]
            attT = [perp.tile([128, SH], BF16, name=f"attT{pb}",
                              tag=f"attT{pb}")
                    for pb in range(KB_D)]
            # pt: exp(score) tiles; 2 heads in flight
            pt = [[perp.tile([128, SH], BF16, name=f"pt{s}_{kb}",
                             tag=f"pt{s}_{kb}")
                   for kb in range(KB)] for s in range(2)]

            # ---- phase 1: qT = (wqT.T @ xq + bq) * s ---------------------
            for pb in range(KB_D):
                ps = psp.tile([128, SH], F32, name="mmq", tag="sc", bufs=3)
                for kb in range(KB_D):
                    for nb in range(2):
                        nc.tensor.matmul(
                            ps[:, nb * 512:(nb + 1) * 512],
                            lhsT=wq[kb][:, pb * 128:(pb + 1) * 128],
                            rhs=xq[kb][:, nb * 512:(nb + 1) * 512],
                            start=(kb == 0),
                            stop=(kb == KB_D - 1),
                        )
                nc.vector.tensor_scalar(
                    out=qT[pb][:],
                    in0=ps[:],
                    scalar1=bqs[:, pb:pb + 1],
                    scalar2=sq[:, pb:pb + 1],
                    op0=ALU.add,
                    op1=ALU.mult,
                )

            # ---- phase 2: kT = wkT.T @ xkv + bk --------------------------
            kchunks = []
            off = 0
            while off < KVP:
                w = min(1024, KVP - off)
                kchunks.append((off, w))
                off += w
            for pb in range(KB_D):
                for off, w in kchunks:
                    ps = psp.tile([128, SH], F32, name="mmk", tag="sc", bufs=3)
                    for kb in range(KB_D):
                        c = 0
                        while c < w:
                            cw = min(512, w - c)
                            nc.tensor.matmul(
                                ps[:, c:c + cw],
                                lhsT=wk[kb][:, pb * 128:(pb + 1) * 128],
                                rhs=xkv[kb][:, off + c:off + c + cw],
                                start=(kb == 0),
                                stop=(kb == KB_D - 1),
                            )
                            c += cw
                    nc.vector.tensor_scalar(
                        out=kT[pb][:, off:off + w],
                        in0=ps[:, :w],
                        scalar1=bks[:, pb:pb + 1],
                        scalar2=None,
                        op0=ALU.add,
                    )

            # ---- phase 3: v' = [x @ wvT | 256] ---------------------------
            for sb in range(KB):
                ps = psp.tile([128, SH], F32, name="mmv", tag="sc", bufs=3)
                for kb in range(KB_D):
                    for lo, hi in ((0, 512), (512, 768)):
                        nc.tensor.matmul(
                            ps[:, lo:hi],
                            lhsT=xkv[kb][:, sb * 128:(sb + 1) * 128],
                            rhs=wv[kb][:, lo:hi],
                            start=(kb == 0),
                            stop=(kb == KB_D - 1),
                        )
                dst = vp[sb].rearrange("p (h c) -> p h c", c=65)[:, :, 0:64]
                nc.scalar.copy(
                    dst, ps[:, :D_MODEL].rearrange("p (h c) -> p h c", c=64))
                ones_col = vp[sb].rearrange("p (h c) -> p h c", c=65)[:, :, 64:65]
                nc.vector.memset(ones_col, 256.0)

            # ---- phase 4: attention, software-pipelined per head ---------
            op_tiles = [None, None]   # live op psum per pipeline slot

            def emit_scores(h):
                s = h % 2
                pb, po = h // 2, 64 * (h % 2)
                for kb in range(KB):
                    sc = psp.tile([128, SH], F32, name="sc", tag="sc", bufs=3)
                    for nb in range(2):
                        nc.tensor.matmul(
                            sc[:, nb * 512:(nb + 1) * 512],
                            lhsT=kT[pb][po:po + 64, kb * 128:(kb + 1) * 128],
                            rhs=qT[pb][po:po + 64, nb * 512:(nb + 1) * 512],
                            start=True,
                            stop=True,
                        )
                    nc.scalar.activation(
                        pt[s][kb][:],
                        sc[:],
                        AF.Exp,
                        bias=kbias[:, kb * NUM_HEADS + h:kb * NUM_HEADS + h + 1],
                        scale=1.0,
                    )

            rrows = [None] * NUM_HEADS
            att_us = [None] * NUM_HEADS

            def emit_pv(h):
                s = h % 2
                op = psp.tile([65, SH], F32, name="op", tag="op", bufs=1)
                for kb in range(KB):
                    for nb in range(2):
                        nc.tensor.matmul(
                            op[:, nb * 512:(nb + 1) * 512],
                            lhsT=vp[kb][:, h * 65:h * 65 + 65],
                            rhs=pt[s][kb][:, nb * 512:(nb + 1) * 512],
                            start=(kb == 0),
                            stop=(kb == KB - 1),
                        )
                # drain op quickly (frees the psum slot): numerators to SBUF
                # via DVE, den row via DMA; the reciprocal runs off the SBUF
                # copy so it never holds the psum buffer. The broadcast and
                # multiply happen one head later so the PE never waits on it.
                den = rbp.tile([1, SH], F32, name="den", tag="den", bufs=3)
                nc.vector.tensor_copy(den[:], op[64:65, :])
                att_u = rbp.tile([64, SH], BF16, name="att_u", tag="att_u",
                                 bufs=3)
                nc.vector.tensor_copy(att_u[:], op[0:64, :])
                rrow = rbp.tile([1, SH], F32, name="rrow", tag="rrow", bufs=3)
                nc.vector.reciprocal(rrow[:], den[:])
                rrows[h], att_us[h] = rrow, att_u

            def emit_norm(h):
                pb, po = h // 2, 64 * (h % 2)
                bc = psp.tile([128, SH], F32, name="bc", tag="sc", bufs=3)
                for nb in range(2):
                    nc.tensor.matmul(
                        bc[0:64, nb * 512:(nb + 1) * 512],
                        lhsT=onescol[0:1, 0:64],
                        rhs=rrows[h][0:1, nb * 512:(nb + 1) * 512],
                        start=True,
                        stop=True,
                    )
                if po == 0:
                    nc.vector.tensor_mul(
                        attT[pb][0:64, :], att_us[h][:], bc[0:64, :])
                else:
                    stage = rbp.tile([64, SH], BF16, name="nstage",
                                     tag="nstage", bufs=2)
                    nc.vector.tensor_mul(stage[:], att_us[h][:], bc[0:64, :])
                    nc.sync.dma_start(out=attT[pb][64:128, :], in_=stage[:])

            emit_scores(0)
            emit_scores(1)
            emit_pv(0)
            for h in range(2, NUM_HEADS):
                emit_scores(h)
                emit_pv(h - 1)
                emit_norm(h - 2)
            emit_pv(NUM_HEADS - 1)
            emit_norm(NUM_HEADS - 2)
            emit_norm(NUM_HEADS - 1)

            # ---- phase 5: out = attT.T @ woT -----------------------------
            for sb in range(SH // 128):
                ps = psp.tile([128, SH], F32, name="mmo", tag="sc", bufs=3)
                for pb in range(KB_D):
                    for lo, hi in ((0, 512), (512, 768)):
                        nc.tensor.matmul(
                            ps[:, lo:hi],
                            lhsT=attT[pb][:, sb * 128:(sb + 1) * 128],
                            rhs=woT[pb][:, lo:hi],
                            start=(pb == 0),
                            stop=(pb == KB_D - 1),
                        )
                ob = obp.tile([128, D_MODEL], F8, name="ob", tag="ob")
                # psum = att * (256*Wo) = 256*out_true; store 128*out_true
                nc.scalar.activation(ob[:], ps[:, :D_MODEL], AF.Identity,
                                     bias=0.0, scale=0.5)
                nc.sync.dma_start(
                    out=out_d[sb * 128:(sb + 1) * 128, :], in_=ob[:])

    _split_wide_waits(nc)
    return nc


def _plan_kvp(mask):
    counts = [int((mask[b] != 0).sum()) for b in range(B)]
    kvp = max(128, int(math.ceil(max(counts) / 128.0)) * 128)
    return min(kvp, S)


def _prep_core_inputs(x, mask, Wq, bq, Wk, bk, Wv, bv, Wo, bo, temperature,
                      KVP):
    """Build the 8 per-core input dicts (disjoint fp8 shards, no gathers)."""
    import ml_dtypes

    f8 = ml_dtypes.float8_e3m4
    KB = KVP // 128
    scale = (np.asarray(temperature, np.float64)
             / math.sqrt(D_QKV)).astype(np.float32)       # [12]

    pack = (np.concatenate([Wq.T, Wk.T, Wv.T, Wo.T], axis=0) * 256).astype(f8)

    pidx = np.arange(128)
    bqs = (256.0 * bq.reshape(KB_D, 128).T).astype(np.float32)
    bqs = np.ascontiguousarray(bqs)                       # [128, 6]
    bks = np.ascontiguousarray(
        (256.0 * bk.reshape(KB_D, 128).T).astype(np.float32))
    # feature f = pb*128 + p belongs to head f//64
    heads = (pidx[:, None] + 128 * np.arange(KB_D)[None, :]) // D_QKV
    sqm = np.ascontiguousarray(
        (scale[heads] / 65536.0).astype(np.float32))      # [128, 6]

    in_maps = []
    per_batch = {}
    for b in range(B):
        live = np.nonzero(np.asarray(mask[b]) != 0)[0]
        xkv = np.zeros((D_MODEL, KVP), np.float32)
        xkv[:, :live.size] = x[b].T[:, live]
        kmask = np.zeros(KVP, np.float32)
        kmask[:live.size] = 1.0
        # kbias[p, kb*12 + h] = (kmask-1) * 1e9 * scale[h]
        km = kmask.reshape(KB, 128)                       # [KB, 128]
        kbias = ((km[:, :, None] - 1.0) * (1e9 * scale)[None, None, :])
        kbias = np.ascontiguousarray(
            kbias.transpose(1, 0, 2).reshape(128, KB * NUM_HEADS)
        ).astype(np.float32)
        per_batch[b] = (xkv.astype(f8), kbias)

    for core in range(N_CORES):
        b, g = core // 2, core % 2
        xkv8, kbias = per_batch[b]
        in_maps.append({
            "xq": np.ascontiguousarray(
                x[b].T[:, g * SH:(g + 1) * SH]).astype(f8),
            "xkv": xkv8,
            "wp": pack,
            "bqs": bqs, "bks": bks, "sq": sqm,
            "kbias": kbias,
        })
    return in_maps


def kernel(x, mask, Wq, bq, Wk, bk, Wv, bv, Wo, bo, temperature, **kw):
    x = np.asarray(x, np.float32)
    mask = np.asarray(mask)
    args = [np.asarray(a, np.float32) for a in (Wq, bq, Wk, bk, Wv, bv, Wo, bo)]
    temperature = np.asarray(temperature, np.float32)

    KVP = _plan_kvp(mask)
    if KVP not in _PROGRAMS:
        _PROGRAMS[KVP] = _build_program(KVP)
    nc = _PROGRAMS[KVP]

    in_maps = _prep_core_inputs(x, mask, *args, temperature, KVP)
    res = run_bass_kernel_spmd(nc, in_maps, core_ids=list(range(N_CORES)))

    Wo_f, bo_f, bv_f = args[6], args[7], args[5]
    hostvec = bv_f @ Wo_f.T + bo_f   # bv contributes a fixed row vector
    out = np.empty((B, S, D_MODEL), np.float32)
    for b in range(B):
        for g in range(2):
            out[b, g * SH:(g + 1) * SH] = (
                res.results[2 * b + g]["out"].astype(np.float32) * (1 / 128)
                + hostvec)
    return out
